# revision 1
# baseline (speedup 1.0000x reference)
"""Trainium2 Bass kernel for BSplineNN: cubic B-spline evaluation.

out[b, c] = sum_i coefficients[b, i, c] * N_{i,3}(x_b),  x_b = inpce[b, 0]

Key property exploited: a cubic B-spline basis at a single point has at most
4 non-zero entries (rows i0..i0+3 with i0 = clamp(searchsorted(t, x) - 4, 0, 60)
= #{j in [4, 64): t[j] <= x} since the knots are sorted).
So instead of reading all 64 coefficient rows per batch (268 MB total), we:
  1. compute the knot-interval index on-chip from the (small) knots tensor,
  2. indirect-DMA-gather only the 4 relevant coefficient rows (4 KB/batch)
     and the 8 relevant knots per batch,
  3. run the Cox-de Boor recurrence on the 8-knot window (sizes 7->6->5->4),
     giving exactly the 4 non-zero basis weights,
  4. weighted-sum the 4 gathered rows.

Sharding: pure data parallel, batch dim split across 8 cores (512 each).
Within a core, batch b = 4*p + g (p = partition 0..127, g = group 0..3).
"""

import numpy as np

import concourse.bacc as bacc
import concourse.bass as bass
import concourse.mybir as mybir
import concourse.tile as tile
from concourse.bass_utils import run_bass_kernel_spmd

B, N, C, T = 4096, 64, 256, 68   # batch, coef rows, channels, knots
K = 3                            # cubic
NCORES = 8
BC = B // NCORES                 # 512 batches per core
P = 128                          # partitions
G = BC // P                      # 4 batch-groups per partition
WROWS = K + 1                    # 4 gathered coef rows per batch
WKNOTS = 2 * K + 2               # 8 gathered knots per batch
F32 = mybir.dt.float32
I32 = mybir.dt.int32


def _emit(tc, nc, coef, knots, inpce, out):
    with tc.tile_pool(name="sb", bufs=1) as sb:
        # ---- load knots + x (layout b = 4p + g) ----
        kt = sb.tile([P, G, T], F32)
        nc.sync.dma_start(out=kt[:], in_=knots.rearrange("(p g) t -> p g t", g=G))
        xt = sb.tile([P, G], F32)
        nc.scalar.dma_start(out=xt[:], in_=inpce.rearrange("(p g) o -> p (g o)", g=G))

        # ---- interval index ----
        # i0 = clamp(#{j in [0,68): t[j] <= x} - 4, 0, 60) is identically
        # #{j in [4,64): t[j] <= x} (knots sorted), so compare only the middle
        # 60 knots and skip the clamp ops entirely.
        ind = sb.tile([P, G, N - WROWS], F32)
        nc.vector.tensor_tensor(out=ind[:],
                                in0=xt[:].to_broadcast([P, G, N - WROWS]),
                                in1=kt[:][:, :, WROWS:N],
                                op=mybir.AluOpType.is_ge)
        i0f = sb.tile([P, G], F32)
        nc.vector.reduce_sum(out=i0f[:], in_=ind[:], axis=mybir.AxisListType.X)
        i0i = sb.tile([P, G], I32)
        nc.vector.tensor_copy(out=i0i[:], in_=i0f[:])

        # ---- gather indices ----
        # coef rows: flat row index into [BC*N, C] = (4p+g)*N + i0
        bi = sb.tile([P, G], I32)
        nc.gpsimd.iota(out=bi[:], pattern=[[N, G]], base=0, channel_multiplier=N * G)
        gidx = sb.tile([P, G], I32)
        nc.vector.tensor_tensor(out=gidx[:], in0=bi[:], in1=i0i[:],
                                op=mybir.AluOpType.add)
        # knot window: flat element index into [BC*T] = (4p+g)*T + i0
        bik = sb.tile([P, G], I32)
        nc.gpsimd.iota(out=bik[:], pattern=[[T, G]], base=0, channel_multiplier=T * G)
        kidx = sb.tile([P, G], I32)
        nc.vector.tensor_tensor(out=kidx[:], in0=bik[:], in1=i0i[:],
                                op=mybir.AluOpType.add)

        # HW indirect DMA consumes ONE index per partition and gathers
        # out_free_size contiguous elements per partition, so issue one
        # gather per batch-group g with a [P, 1] index slice.
        # SWDGE emission order (the serial Pool engine AND the single SWDGE
        # DMA queue are serial resources): all four tiny knot-window gathers
        # first — their data jumps the queue ahead of the 512 KB coef blocks,
        # so the basis is done early and each group's contraction follows its
        # own coef block as it streams in.
        kw = sb.tile([P, G, WKNOTS], F32)
        gt = sb.tile([P, G, WROWS * C], F32)
        order = []
        for g in range(G):
            order.append(nc.gpsimd.indirect_dma_start(
                out=kw[:][:, g, :], out_offset=None,
                in_=knots.rearrange("b (t o) -> (b t) o", o=1),
                in_offset=bass.IndirectOffsetOnAxis(
                    ap=kidx[:][:, g:g + 1], axis=0)))
        for g in range(G):
            order.append(nc.gpsimd.indirect_dma_start(
                out=gt[:][:, g, :], out_offset=None,
                in_=coef.rearrange("b n c -> (b n) c"),
                in_offset=bass.IndirectOffsetOnAxis(
                    ap=gidx[:][:, g:g + 1], axis=0)))
        for a, b in zip(order[1:], order):
            tile.add_dep_helper(a.ins, b.ins, sync=False,
                                reason="SWDGE emission order")

        # ---- windowed Cox-de Boor on kw, in two independent halves ----
        # Uses w1[i] = U[i], w2[i] = 1 - U[i+1] with U[j] = (x-t[j])/(t[j+kk]-t[j]):
        #   Bnew[i] = U[i]*B[i] + B[i+1] - U[i+1]*B[i+1]   (7 ops per level)
        indw = sb.tile([P, G, WKNOTS], F32)
        xmt = sb.tile([P, G, WKNOTS], F32)
        levels = [sb.tile([P, G, WKNOTS - 1 - kk], F32, name=f"lvl{kk}")
                  for kk in range(K + 1)]
        HG = 2  # groups per half
        for h in (0, 1):
            gs = slice(HG * h, HG * h + HG)
            kwh = kw[:][:, gs, :]
            xb8 = xt[:][:, gs].to_broadcast([P, HG, WKNOTS])
            nc.vector.tensor_tensor(out=indw[:][:, gs, :], in0=xb8, in1=kwh,
                                    op=mybir.AluOpType.is_ge)
            nc.vector.tensor_tensor(out=xmt[:][:, gs, :], in0=xb8, in1=kwh,
                                    op=mybir.AluOpType.subtract)
            nc.vector.tensor_tensor(
                out=levels[0][:][:, gs, :],
                in0=indw[:][:, gs, 0:WKNOTS - 1],
                in1=indw[:][:, gs, 1:WKNOTS], op=mybir.AluOpType.subtract)
            for kk in range(1, K + 1):
                L = WKNOTS - 1 - kk
                prev = levels[kk - 1][:][:, gs, :]
                d1 = sb.tile([P, HG, L + 1], F32, tag=f"d1_{kk}_{h}")
                u1 = sb.tile([P, HG, L + 1], F32, tag=f"u1_{kk}_{h}")
                a1 = sb.tile([P, HG, L], F32, tag=f"a1_{kk}_{h}")
                t2 = sb.tile([P, HG, L], F32, tag=f"t2_{kk}_{h}")
                nc.vector.tensor_tensor(out=d1[:], in0=kwh[:, :, kk:kk + L + 1],
                                        in1=kwh[:, :, 0:L + 1],
                                        op=mybir.AluOpType.subtract)
                nc.vector.reciprocal(out=u1[:], in_=d1[:])
                nc.vector.tensor_tensor(out=u1[:], in0=xmt[:][:, gs, 0:L + 1],
                                        in1=u1[:], op=mybir.AluOpType.mult)
                nc.vector.tensor_tensor(out=a1[:], in0=u1[:][:, :, 0:L],
                                        in1=prev[:, :, 0:L],
                                        op=mybir.AluOpType.mult)
                nc.vector.tensor_tensor(out=t2[:], in0=u1[:][:, :, 1:L + 1],
                                        in1=prev[:, :, 1:1 + L],
                                        op=mybir.AluOpType.mult)
                nc.vector.tensor_tensor(out=t2[:], in0=prev[:, :, 1:1 + L],
                                        in1=t2[:], op=mybir.AluOpType.subtract)
                nc.vector.tensor_tensor(out=levels[kk][:][:, gs, :], in0=a1[:],
                                        in1=t2[:], op=mybir.AluOpType.add)
        wts = levels[K]  # [P, G, 4] basis weights for rows i0..i0+3

        # ---- weighted sum of the 4 gathered rows, per group (pipelines with
        # the coef gathers; wts[:, g, d] is a [P,1] per-partition scalar).
        # d=0 multiply runs on the otherwise-idle ACT engine; the stt chain
        # stays on DVE; each group's result is stored as soon as it's done
        # (alternating the two HWDGE rings). ----
        gtv = gt[:].rearrange("p g (d c) -> p g d c", d=WROWS)
        outv = out.rearrange("(p g) c -> p g c", g=G)
        acc = sb.tile([P, G, C], F32)
        for g in range(G):
            nc.vector.tensor_scalar_mul(out=acc[:][:, g, :], in0=gtv[:, g, 0, :],
                                        scalar1=wts[:][:, g, 0:1])
            for d in range(1, WROWS):
                nc.vector.scalar_tensor_tensor(
                    out=acc[:][:, g, :], in0=gtv[:, g, d, :],
                    scalar=wts[:][:, g, d:d + 1], in1=acc[:][:, g, :],
                    op0=mybir.AluOpType.mult, op1=mybir.AluOpType.add)
            eng = nc.sync if g % 2 == 0 else nc.scalar
            eng.dma_start(out=outv[:, g, :], in_=acc[:][:, g, :])


def build_nc(reps=1):
    nc = bacc.Bacc("TRN2", target_bir_lowering=False, debug=False,
                   num_devices=NCORES)
    coef = nc.dram_tensor("coefficients", [BC, N, C], F32, kind="ExternalInput")
    knots = nc.dram_tensor("knots", [BC, T], F32, kind="ExternalInput")
    inpce = nc.dram_tensor("inpce", [BC, 1], F32, kind="ExternalInput")
    out = nc.dram_tensor("out", [BC, C], F32, kind="ExternalOutput")
    with tile.TileContext(nc) as tc:
        for _ in range(reps):
            _emit(tc, nc, coef.ap(), knots.ap(), inpce.ap(), out.ap())
    nc.compile()
    return nc


def build_nc_loop(trip):
    """Kernel body wrapped in a hardware For_i loop — for benchmarking only."""
    nc = bacc.Bacc("TRN2", target_bir_lowering=False, debug=False,
                   num_devices=NCORES)
    coef = nc.dram_tensor("coefficients", [BC, N, C], F32, kind="ExternalInput")
    knots = nc.dram_tensor("knots", [BC, T], F32, kind="ExternalInput")
    inpce = nc.dram_tensor("inpce", [BC, 1], F32, kind="ExternalInput")
    out = nc.dram_tensor("out", [BC, C], F32, kind="ExternalOutput")
    with tile.TileContext(nc) as tc:
        with tc.For_i(0, trip, 1):
            _emit(tc, nc, coef.ap(), knots.ap(), inpce.ap(), out.ap())
    nc.compile()
    return nc


_NC_CACHE = None


def kernel(coefficients, knots, inpce, **run_kwargs):
    global _NC_CACHE
    if _NC_CACHE is None:
        _NC_CACHE = build_nc()
    nc = _NC_CACHE
    coefficients = np.ascontiguousarray(coefficients, dtype=np.float32)
    knots = np.ascontiguousarray(knots, dtype=np.float32)
    inpce = np.ascontiguousarray(inpce, dtype=np.float32)
    in_maps = []
    for k in range(NCORES):
        s = slice(k * BC, (k + 1) * BC)
        in_maps.append({"coefficients": coefficients[s],
                        "knots": knots[s],
                        "inpce": inpce[s]})
    res = run_bass_kernel_spmd(nc, in_maps, core_ids=list(range(NCORES)),
                               **run_kwargs)
    out = np.concatenate([res.results[k]["out"] for k in range(NCORES)], axis=0)
    if run_kwargs:
        return out, res
    return out



# revision 3
# speedup vs baseline: 1.0755x; 1.0755x over previous
"""Trainium2 Bass kernel v2 for BSplineNN: cubic B-spline evaluation.

out[b, c] = sum_i coefficients[b, i, c] * N_{i,3}(x_b),  x_b = inpce[b, 0]

Same math as v1 (4 non-zero cubic basis entries; indirect-gather the 4
coefficient rows + the 8-knot window per batch), restructured for engine
balance:
  - ONE merged indirect DMA for all 512 knot windows (multi-index offset AP)
    instead of 4 -> saves ~3us of Pool DGE time.
  - Coefficient gathers split per `coef_split` so compute pipelines behind
    the gather stream.
  - Cox-de Boor merged across all 4 groups: batched divided-differences
    D[kk,j] = t[j+kk+1]-t[j] in one op (overlapping strided AP), one
    reciprocal, one U = xmt*R, then 3 ops per level via
    Bnew[i] = a[i] + (B[i+1] - a[i+1]), a = U.*B.
  - Index arithmetic (iota, +i0) and level-0 indicator on gpsimd.
  - Weighted sum: fused scalar_tensor_tensor chain per group on DVE
    (optionally multiplies on ACT for trailing groups).

Sharding: pure data parallel, batch dim split across 8 cores (512 each).
Within a core, batch b = 4*p + g (p = partition 0..127, g = group 0..3).
"""

import numpy as np

import concourse.bacc as bacc
import concourse.bass as bass
import concourse.mybir as mybir
import concourse.tile as tile
from concourse.bass_utils import run_bass_kernel_spmd

B, N, C, T = 4096, 64, 256, 68   # batch, coef rows, channels, knots
K = 3                            # cubic
NCORES = 8
BC = B // NCORES                 # 512 batches per core
P = 128                          # partitions
G = BC // P                      # 4 batch-groups per partition
WROWS = K + 1                    # 4 gathered coef rows per batch
WKNOTS = 2 * K + 2               # 8 gathered knots per batch
KWPAD = G * WKNOTS + 4           # kw tile padded for overlapping D reads
F32 = mybir.dt.float32
I32 = mybir.dt.int32

# NOTE: multi-index indirect DMA (offset AP with >1 index per partition) is
# silently broken on HW (only the first index per partition is honored), so
# split_kw must stay True and coef_split must stay (1,1,1,1).
DEFAULTS = dict(
    coef_split=(1, 1, 1, 1),  # how the 4 group coef gathers are merged
    act_groups=2,             # trailing groups whose wsum mults go on ACT
    i0_on_pool=False,         # reduce for i0 on gpsimd instead of DVE
    split_kw=True,            # one [P,1]-index window gather per group
    idx_on_dve=True,          # index copy/adds on DVE instead of gpsimd
    batched_d=True,           # batched overlapping-AP D/U (else per-kk slices)
    debug_dump=None,          # 'i0' | 'kw' | 'wts': dump intermediate to out
    no_kw=False,              # timing probe: skip kw gathers + CdB
    no_wsum=False,            # timing probe: store gathered rows directly
    gt_bf16=False,            # cast coef gather to bf16 during SWDGE
    nodep=True,               # skip the explicit SWDGE emission-order chain
)


def _strided(a, dims, extra_offset):
    """Overlapping strided free-axis view of a 2-D [P, F] AP.

    dims: list of [stride, count] free dims (innermost last)."""
    b = a.copy()
    V = type(b.ap)
    b.ap = V([list(a.ap[0])] + [list(d) for d in dims])
    b.offset = a.offset + extra_offset
    return b


def _emit_hoisted(tc, nc, hp):
    """Loop-invariant index bases: flat row/knot base per (p, g)."""
    bi = hp.tile([P, G], I32, tag="bi")
    nc.gpsimd.iota(out=bi[:], pattern=[[N, G]], base=0, channel_multiplier=N * G)
    bik = hp.tile([P, G], I32, tag="bik")
    nc.gpsimd.iota(out=bik[:], pattern=[[T, G]], base=0, channel_multiplier=T * G)
    return bi, bik


def _emit(tc, nc, sb, hoisted, coef, knots, inpce, out,
          coef_split=(1, 1, 1, 1), act_groups=0, i0_on_pool=False,
          split_kw=False, idx_on_dve=False, batched_d=True, debug_dump=None,
          no_kw=False, no_wsum=False, gt_bf16=False, nodep=False):
    if sb is None:
        with tc.tile_pool(name="sbi", bufs=1) as sbi:
            hoisted = _emit_hoisted(tc, nc, sbi)
            return _emit(tc, nc, sbi, hoisted, coef, knots, inpce, out,
                         coef_split=coef_split, act_groups=act_groups,
                         i0_on_pool=i0_on_pool, split_kw=split_kw,
                         idx_on_dve=idx_on_dve, batched_d=batched_d,
                         debug_dump=debug_dump, no_kw=no_kw,
                         no_wsum=no_wsum, gt_bf16=gt_bf16)
    bi, bik = hoisted
    eng_idx = nc.vector if idx_on_dve else nc.gpsimd

    # ---- load the 60 middle knots (all i0 needs) + x (layout b = 4p+g) ----
    NM = N - WROWS  # 60
    kt = sb.tile([P, G, NM], F32, tag="kt")
    nc.sync.dma_start(
        out=kt[:],
        in_=knots.rearrange("(p g) t -> p g t", g=G)[:, :, WROWS:N])
    xt = sb.tile([P, G], F32, tag="xt")
    nc.scalar.dma_start(out=xt[:], in_=inpce.rearrange("(p g) o -> p (g o)", g=G))

    # ---- interval index: i0 = #{j in [4,64): t[j] <= x} in [0, 60] ----
    # (is_ge only exists on DVE; Pool can do the reduce)
    eng_red = nc.gpsimd if i0_on_pool else nc.vector
    ind = sb.tile([P, G, NM], F32, tag="ind")
    nc.vector.tensor_tensor(out=ind[:],
                            in0=xt[:].to_broadcast([P, G, NM]),
                            in1=kt[:],
                            op=mybir.AluOpType.is_ge)
    i0f = sb.tile([P, G], F32, tag="i0f")
    eng_red.reduce_sum(out=i0f[:], in_=ind[:], axis=mybir.AxisListType.X)
    i0i = sb.tile([P, G], I32, tag="i0i")
    eng_idx.tensor_copy(out=i0i[:], in_=i0f[:])

    # ---- gather indices ----
    gidx = sb.tile([P, G], I32, tag="gidx")
    eng_idx.tensor_tensor(out=gidx[:], in0=bi[:], in1=i0i[:],
                          op=mybir.AluOpType.add)
    kidx = sb.tile([P, G], I32, tag="kidx")
    eng_idx.tensor_tensor(out=kidx[:], in0=bik[:], in1=i0i[:],
                          op=mybir.AluOpType.add)

    # ---- merged gathers ----
    kw = sb.tile([P, KWPAD], F32, tag="kw")
    nc.gpsimd.memset(kw[:][:, G * WKNOTS:KWPAD], 0.0)
    kwv = kw[:][:, 0:G * WKNOTS].rearrange("p (g w) -> p g w", g=G)
    gt = sb.tile([P, G, WROWS * C],
                 mybir.dt.bfloat16 if gt_bf16 else F32, tag="gt")

    def kw_gather(g):
        return nc.gpsimd.indirect_dma_start(
            out=kwv[:, g, :], out_offset=None,
            in_=knots.rearrange("b (t o) -> (b t) o", o=1),
            in_offset=bass.IndirectOffsetOnAxis(ap=kidx[:][:, g:g + 1],
                                                axis=0))

    def coef_gather(g):
        return nc.gpsimd.indirect_dma_start(
            out=gt[:][:, g, :], out_offset=None,
            in_=coef.rearrange("b n c -> (b n) c"),
            in_offset=bass.IndirectOffsetOnAxis(ap=gidx[:][:, g:g + 1],
                                                axis=0))

    # Queue order: coef g0 first (its data streams while the basis is
    # computed), then the 4 tiny window gathers (basis inputs), then the
    # remaining coef groups.
    order = [coef_gather(0)]
    if not no_kw:
        order += [kw_gather(g) for g in range(G)]
    order += [coef_gather(g) for g in range(1, G)]
    if not nodep:
        for a, b in zip(order[1:], order):
            tile.add_dep_helper(a.ins, b.ins, sync=False,
                                reason="SWDGE emission order")

    gtv = gt[:].rearrange("p g (d c) -> p g d c", d=WROWS)
    outv = out.rearrange("(p g) c -> p g c", g=G)
    if no_wsum:
        for g in range(G):
            eng = nc.sync if g % 2 == 0 else nc.scalar
            src = (gt[:][:, g, 0:2 * C].bitcast(F32) if gt_bf16
                   else gtv[:, g, 0, :])
            eng.dma_start(out=outv[:, g, :], in_=src)
        return
    if no_kw:
        # fake per-batch weights (timing only): reuse xt as scalars
        acc = sb.tile([P, G, C], F32, tag="acc")
        for g in range(G):
            nc.vector.tensor_scalar_mul(out=acc[:][:, g, :],
                                        in0=gtv[:, g, 0, :],
                                        scalar1=xt[:][:, g:g + 1])
            for d in range(1, WROWS):
                nc.vector.scalar_tensor_tensor(
                    out=acc[:][:, g, :], in0=gtv[:, g, d, :],
                    scalar=xt[:][:, g:g + 1], in1=acc[:][:, g, :],
                    op0=mybir.AluOpType.mult, op1=mybir.AluOpType.add)
            eng = nc.sync if g % 2 == 0 else nc.scalar
            eng.dma_start(out=outv[:, g, :], in_=acc[:][:, g, :])
        return

    # ---- merged Cox-de Boor over all G groups ----
    # xmt[j] = x - t[i0+j]; ind8[j] = (x >= t[i0+j]); B0[j] = ind8[j]-ind8[j+1]
    xb8 = xt[:].to_broadcast([P, G, WKNOTS])
    xmt = sb.tile([P, G, WKNOTS], F32, tag="xmt")
    nc.vector.tensor_tensor(out=xmt[:], in0=xb8, in1=kwv,
                            op=mybir.AluOpType.subtract)
    ind8 = sb.tile([P, G, WKNOTS], F32, tag="ind8")
    nc.vector.tensor_tensor(out=ind8[:], in0=xb8, in1=kwv,
                            op=mybir.AluOpType.is_ge)
    B0 = sb.tile([P, G, WKNOTS - 1], F32, tag="B0")
    nc.vector.tensor_tensor(out=B0[:], in0=ind8[:][:, :, 0:WKNOTS - 1],
                            in1=ind8[:][:, :, 1:WKNOTS],
                            op=mybir.AluOpType.subtract)

    # batched divided differences D[g, kk, j] = t[g, j+kk+1] - t[g, j],
    # kk = 0..2 (level kk+1), j = 0..6. hi reads kw flat offsets
    # g*8 + kk + 1 + j <= 33 < KWPAD (pad memset above; garbage lanes of
    # D/R/U are never consumed by the level ops).
    W1 = WKNOTS - 1
    kwf = kw[:]  # [P, KWPAD]
    D = sb.tile([P, G, K, W1], F32, tag="D")
    U = sb.tile([P, G, K, W1], F32, tag="U")
    R = sb.tile([P, G, K, W1], F32, tag="R")
    if batched_d:
        hi = _strided(kwf, [[WKNOTS, G], [1, K], [1, W1]], 1)
        lo = _strided(kwf, [[WKNOTS, G], [0, K], [1, W1]], 0)
        nc.vector.tensor_tensor(out=D[:], in0=hi, in1=lo,
                                op=mybir.AluOpType.subtract)
        nc.vector.reciprocal(out=R[:].rearrange("p g k w -> p (g k w)"),
                             in_=D[:].rearrange("p g k w -> p (g k w)"))
        # U[g, kk, j] = xmt[g, j] * R[g, kk, j]
        xmt_b = _strided(xmt[:], [[WKNOTS, G], [0, K], [1, W1]], 0)
        nc.vector.tensor_tensor(out=U[:], in0=xmt_b, in1=R[:],
                                op=mybir.AluOpType.mult)
    else:
        for kk in range(K):
            # hi[p, g, j] = kw[p, g*8 + kk + 1 + j], j = 0..6
            hi = kwf[:, kk + 1:kk + 1 + G * WKNOTS] \
                .rearrange("p (g w) -> p g w", w=WKNOTS)[:, :, 0:W1]
            nc.vector.tensor_tensor(out=D[:][:, :, kk, :], in0=hi,
                                    in1=kwv[:, :, 0:W1],
                                    op=mybir.AluOpType.subtract)
        nc.vector.reciprocal(out=R[:].rearrange("p g k w -> p (g k w)"),
                             in_=D[:].rearrange("p g k w -> p (g k w)"))
        for kk in range(K):
            nc.vector.tensor_tensor(out=U[:][:, :, kk, :],
                                    in0=xmt[:][:, :, 0:W1],
                                    in1=R[:][:, :, kk, :],
                                    op=mybir.AluOpType.mult)

    # levels: Bk[i] = a[i] + (B[i+1] - a[i+1]),  a = U[kk-1] .* B (width L+1)
    prev = B0
    for kk in range(1, K + 1):
        L = WKNOTS - 1 - kk
        a = sb.tile([P, G, L + 1], F32, tag=f"a{kk}")
        nc.vector.tensor_tensor(out=a[:], in0=U[:][:, :, kk - 1, 0:L + 1],
                                in1=prev[:][:, :, 0:L + 1],
                                op=mybir.AluOpType.mult)
        t2 = sb.tile([P, G, L], F32, tag=f"t2{kk}")
        nc.vector.tensor_tensor(out=t2[:], in0=prev[:][:, :, 1:L + 1],
                                in1=a[:][:, :, 1:L + 1],
                                op=mybir.AluOpType.subtract)
        nxt = sb.tile([P, G, L], F32, tag=f"lvl{kk}")
        nc.vector.tensor_tensor(out=nxt[:], in0=a[:][:, :, 0:L],
                                in1=t2[:], op=mybir.AluOpType.add)
        prev = nxt
    wts = prev  # [P, G, 4]

    # ---- weighted sum of the 4 gathered rows, per group ----
    gtv = gt[:].rearrange("p g (d c) -> p g d c", d=WROWS)
    outv = out.rearrange("(p g) c -> p g c", g=G)
    if debug_dump == "i0":
        nc.sync.dma_start(out=outv[:, :, 0], in_=i0f[:])
        return
    if debug_dump == "kw":
        nc.sync.dma_start(out=outv[:, :, 0:WKNOTS], in_=kwv)
        return
    if debug_dump == "wts":
        nc.sync.dma_start(out=outv[:, :, 0:WROWS], in_=wts[:])
        return
    if debug_dump == "gt":
        nc.sync.dma_start(out=outv[:, :, 0:C], in_=gtv[:, :, 0, :])
        return
    if debug_dump == "gidx":
        gxf = sb.tile([P, G], F32, tag="gxf")
        nc.vector.tensor_copy(out=gxf[:], in_=gidx[:])
        nc.sync.dma_start(out=outv[:, :, 0], in_=gxf[:])
        return
    acc = sb.tile([P, G, C], F32, tag="acc")
    A = act_groups
    GA = G - A  # groups on the DVE STT chain
    for g in range(GA):
        nc.vector.tensor_scalar_mul(out=acc[:][:, g, :],
                                    in0=gtv[:, g, 0, :],
                                    scalar1=wts[:][:, g, 0:1])
        for d in range(1, WROWS):
            nc.vector.scalar_tensor_tensor(
                out=acc[:][:, g, :], in0=gtv[:, g, d, :],
                scalar=wts[:][:, g, d:d + 1], in1=acc[:][:, g, :],
                op0=mybir.AluOpType.mult, op1=mybir.AluOpType.add)
        eng = nc.sync if g % 2 == 0 else nc.scalar
        eng.dma_start(out=outv[:, g, :], in_=acc[:][:, g, :])
    if A:
        # trailing groups: multiplies on ACT, adds folded across groups on DVE
        prod = sb.tile([P, A, WROWS, C], F32, tag="prod", name="prod")
        for g in range(GA, G):
            for d in range(WROWS):
                nc.scalar.activation(out=prod[:][:, g - GA, d, :],
                                     in_=gtv[:, g, d, :],
                                     func=mybir.ActivationFunctionType.Copy,
                                     scale=wts[:][:, g, d:d + 1])
        pv = prod[:]
        nc.vector.tensor_tensor(out=pv[:, :, 0, :], in0=pv[:, :, 0, :],
                                in1=pv[:, :, 1, :], op=mybir.AluOpType.add)
        nc.vector.tensor_tensor(out=pv[:, :, 2, :], in0=pv[:, :, 2, :],
                                in1=pv[:, :, 3, :], op=mybir.AluOpType.add)
        nc.vector.tensor_tensor(out=acc[:][:, GA:G, :], in0=pv[:, :, 0, :],
                                in1=pv[:, :, 2, :], op=mybir.AluOpType.add)
        for g in range(GA, G):
            eng = nc.sync if g % 2 == 0 else nc.scalar
            eng.dma_start(out=outv[:, g, :], in_=acc[:][:, g, :])


def _declare(nc):
    coef = nc.dram_tensor("coefficients", [BC, N, C], F32, kind="ExternalInput")
    knots = nc.dram_tensor("knots", [BC, T], F32, kind="ExternalInput")
    inpce = nc.dram_tensor("inpce", [BC, 1], F32, kind="ExternalInput")
    out = nc.dram_tensor("out", [BC, C], F32, kind="ExternalOutput")
    return coef, knots, inpce, out


def build_nc(reps=1, bufs=1, **flags):
    cfg = {**DEFAULTS, **flags}
    nc = bacc.Bacc("TRN2", target_bir_lowering=False, debug=False,
                   num_devices=NCORES)
    coef, knots, inpce, out = _declare(nc)
    with tile.TileContext(nc) as tc:
        with tc.tile_pool(name="hoist", bufs=1) as hp, \
             tc.tile_pool(name="sb", bufs=bufs) as sb:
            hoisted = _emit_hoisted(tc, nc, hp)
            for _ in range(reps):
                _emit(tc, nc, sb, hoisted, coef.ap(), knots.ap(),
                      inpce.ap(), out.ap(), **cfg)
    nc.compile()
    return nc


def build_nc_loop(trip, unroll=8, bufs=3, pool_inside=False, **flags):
    cfg = {**DEFAULTS, **flags}
    nc = bacc.Bacc("TRN2", target_bir_lowering=False, debug=False,
                   num_devices=NCORES)
    coef, knots, inpce, out = _declare(nc)
    with tile.TileContext(nc) as tc:
        if pool_inside:
            with tc.For_i(0, trip, 1):
                for _ in range(unroll):
                    _emit(tc, nc, None, None, coef.ap(), knots.ap(),
                          inpce.ap(), out.ap(), **cfg)
        else:
            with tc.tile_pool(name="hoist", bufs=1) as hp, \
                 tc.tile_pool(name="sb", bufs=bufs) as sb:
                hoisted = _emit_hoisted(tc, nc, hp)
                with tc.For_i(0, trip, 1):
                    for _ in range(unroll):
                        _emit(tc, nc, sb, hoisted, coef.ap(), knots.ap(),
                              inpce.ap(), out.ap(), **cfg)
    nc.compile()
    return nc


_NC_CACHE = None


def kernel(coefficients, knots, inpce, **run_kwargs):
    global _NC_CACHE
    if _NC_CACHE is None:
        _NC_CACHE = build_nc()
    nc = _NC_CACHE
    coefficients = np.ascontiguousarray(coefficients, dtype=np.float32)
    knots = np.ascontiguousarray(knots, dtype=np.float32)
    inpce = np.ascontiguousarray(inpce, dtype=np.float32)
    in_maps = []
    for k in range(NCORES):
        s = slice(k * BC, (k + 1) * BC)
        in_maps.append({"coefficients": coefficients[s],
                        "knots": knots[s],
                        "inpce": inpce[s]})
    res = run_bass_kernel_spmd(nc, in_maps, core_ids=list(range(NCORES)),
                               **run_kwargs)
    out = np.concatenate([res.results[k]["out"] for k in range(NCORES)], axis=0)
    if run_kwargs:
        return out, res
    return out


# revision 4
# speedup vs baseline: 1.1303x; 1.0509x over previous
"""Trainium2 Bass kernel v2 for BSplineNN: cubic B-spline evaluation.

out[b, c] = sum_i coefficients[b, i, c] * N_{i,3}(x_b),  x_b = inpce[b, 0]

Same math as v1 (4 non-zero cubic basis entries; indirect-gather the 4
coefficient rows + the 8-knot window per batch), restructured for engine
balance:
  - ONE merged indirect DMA for all 512 knot windows (multi-index offset AP)
    instead of 4 -> saves ~3us of Pool DGE time.
  - Coefficient gathers split per `coef_split` so compute pipelines behind
    the gather stream.
  - Cox-de Boor merged across all 4 groups: batched divided-differences
    D[kk,j] = t[j+kk+1]-t[j] in one op (overlapping strided AP), one
    reciprocal, one U = xmt*R, then 3 ops per level via
    Bnew[i] = a[i] + (B[i+1] - a[i+1]), a = U.*B.
  - Index arithmetic (iota, +i0) and level-0 indicator on gpsimd.
  - Weighted sum: fused scalar_tensor_tensor chain per group on DVE
    (optionally multiplies on ACT for trailing groups).

Sharding: pure data parallel, batch dim split across 8 cores (512 each).
Within a core, batch b = 4*p + g (p = partition 0..127, g = group 0..3).
"""

import numpy as np

import concourse.bacc as bacc
import concourse.bass as bass
import concourse.mybir as mybir
import concourse.tile as tile
from concourse.bass_utils import run_bass_kernel_spmd

B, N, C, T = 4096, 64, 256, 68   # batch, coef rows, channels, knots
K = 3                            # cubic
NCORES = 8
BC = B // NCORES                 # 512 batches per core
P = 128                          # partitions
G = BC // P                      # 4 batch-groups per partition
WROWS = K + 1                    # 4 gathered coef rows per batch
WKNOTS = 2 * K + 2               # 8 gathered knots per batch
KWPAD = G * WKNOTS + 4           # kw tile padded for overlapping D reads
F32 = mybir.dt.float32
I32 = mybir.dt.int32

# NOTE: multi-index indirect DMA (offset AP with >1 index per partition) is
# silently broken on HW (only the first index per partition is honored), so
# split_kw must stay True and coef_split must stay (1,1,1,1).
DEFAULTS = dict(
    coef_split=(1, 1, 1, 1),  # how the 4 group coef gathers are merged
    act_groups=2,             # trailing groups whose wsum mults go on ACT
    i0_on_pool=False,         # reduce for i0 on gpsimd instead of DVE
    split_kw=True,            # one [P,1]-index window gather per group
    idx_on_dve=True,          # index copy/adds on DVE instead of gpsimd
    batched_d=True,           # batched overlapping-AP D/U (else per-kk slices)
    debug_dump=None,          # 'i0' | 'kw' | 'wts': dump intermediate to out
    no_kw=False,              # timing probe: skip kw gathers + CdB
    no_wsum=False,            # timing probe: store gathered rows directly
    gt_bf16=False,            # cast coef gather to bf16 during SWDGE
    nodep=True,               # skip the explicit SWDGE emission-order chain
)


def _strided(a, dims, extra_offset):
    """Overlapping strided free-axis view of a 2-D [P, F] AP.

    dims: list of [stride, count] free dims (innermost last)."""
    b = a.copy()
    V = type(b.ap)
    b.ap = V([list(a.ap[0])] + [list(d) for d in dims])
    b.offset = a.offset + extra_offset
    return b


def _emit_hoisted(tc, nc, hp):
    """Loop-invariant index bases: flat row/knot base per (p, g)."""
    bi = hp.tile([P, G], I32, tag="bi")
    nc.gpsimd.iota(out=bi[:], pattern=[[N, G]], base=0, channel_multiplier=N * G)
    bik = hp.tile([P, G], I32, tag="bik")
    nc.gpsimd.iota(out=bik[:], pattern=[[T, G]], base=0, channel_multiplier=T * G)
    return bi, bik


def _emit(tc, nc, sb, hoisted, coef, knots, inpce, out,
          coef_split=(1, 1, 1, 1), act_groups=0, i0_on_pool=False,
          split_kw=False, idx_on_dve=False, batched_d=True, debug_dump=None,
          no_kw=False, no_wsum=False, gt_bf16=False, nodep=False):
    if sb is None:
        with tc.tile_pool(name="sbi", bufs=1) as sbi:
            hoisted = _emit_hoisted(tc, nc, sbi)
            return _emit(tc, nc, sbi, hoisted, coef, knots, inpce, out,
                         coef_split=coef_split, act_groups=act_groups,
                         i0_on_pool=i0_on_pool, split_kw=split_kw,
                         idx_on_dve=idx_on_dve, batched_d=batched_d,
                         debug_dump=debug_dump, no_kw=no_kw,
                         no_wsum=no_wsum, gt_bf16=gt_bf16)
    bi, bik = hoisted
    eng_idx = nc.vector if idx_on_dve else nc.gpsimd

    # ---- load the 60 middle knots (all i0 needs) + x (layout b = 4p+g) ----
    NM = N - WROWS  # 60
    kt = sb.tile([P, G, NM], F32, tag="kt")
    nc.sync.dma_start(
        out=kt[:],
        in_=knots.rearrange("(p g) t -> p g t", g=G)[:, :, WROWS:N])
    xt = sb.tile([P, G], F32, tag="xt")
    nc.scalar.dma_start(out=xt[:], in_=inpce.rearrange("(p g) o -> p (g o)", g=G))

    # ---- interval index: i0 = #{j in [4,64): t[j] <= x} in [0, 60] ----
    # (is_ge only exists on DVE; Pool can do the reduce)
    eng_red = nc.gpsimd if i0_on_pool else nc.vector
    ind = sb.tile([P, G, NM], F32, tag="ind")
    nc.vector.tensor_tensor(out=ind[:],
                            in0=xt[:].to_broadcast([P, G, NM]),
                            in1=kt[:],
                            op=mybir.AluOpType.is_ge)
    i0f = sb.tile([P, G], F32, tag="i0f")
    eng_red.reduce_sum(out=i0f[:], in_=ind[:], axis=mybir.AxisListType.X)
    i0i = sb.tile([P, G], I32, tag="i0i")
    eng_idx.tensor_copy(out=i0i[:], in_=i0f[:])

    # ---- gather indices ----
    gidx = sb.tile([P, G], I32, tag="gidx")
    eng_idx.tensor_tensor(out=gidx[:], in0=bi[:], in1=i0i[:],
                          op=mybir.AluOpType.add)
    kidx = sb.tile([P, G], I32, tag="kidx")
    eng_idx.tensor_tensor(out=kidx[:], in0=bik[:], in1=i0i[:],
                          op=mybir.AluOpType.add)

    # ---- merged gathers ----
    kw = sb.tile([P, KWPAD], F32, tag="kw")
    nc.gpsimd.memset(kw[:][:, G * WKNOTS:KWPAD], 0.0)
    kwv = kw[:][:, 0:G * WKNOTS].rearrange("p (g w) -> p g w", g=G)
    gt = sb.tile([P, G, WROWS * C],
                 mybir.dt.bfloat16 if gt_bf16 else F32, tag="gt")

    def kw_gather(g):
        return nc.gpsimd.indirect_dma_start(
            out=kwv[:, g, :], out_offset=None,
            in_=knots.rearrange("b (t o) -> (b t) o", o=1),
            in_offset=bass.IndirectOffsetOnAxis(ap=kidx[:][:, g:g + 1],
                                                axis=0))

    def coef_gather(g):
        return nc.gpsimd.indirect_dma_start(
            out=gt[:][:, g, :], out_offset=None,
            in_=coef.rearrange("b n c -> (b n) c"),
            in_offset=bass.IndirectOffsetOnAxis(ap=gidx[:][:, g:g + 1],
                                                axis=0))

    # Queue order: coef g0 first (its data streams while the basis is
    # computed), then the 4 tiny window gathers (basis inputs), then the
    # remaining coef groups.
    order = [coef_gather(0)]
    if not no_kw:
        order += [kw_gather(g) for g in range(G)]
    order += [coef_gather(g) for g in range(1, G)]
    if not nodep:
        for a, b in zip(order[1:], order):
            tile.add_dep_helper(a.ins, b.ins, sync=False,
                                reason="SWDGE emission order")

    gtv = gt[:].rearrange("p g (d c) -> p g d c", d=WROWS)
    outv = out.rearrange("(p g) c -> p g c", g=G)
    if no_wsum:
        for g in range(G):
            eng = nc.sync if g % 2 == 0 else nc.scalar
            src = (gt[:][:, g, 0:2 * C].bitcast(F32) if gt_bf16
                   else gtv[:, g, 0, :])
            eng.dma_start(out=outv[:, g, :], in_=src)
        return
    if no_kw:
        # fake per-batch weights (timing only): reuse xt as scalars
        acc = sb.tile([P, G, C], F32, tag="acc")
        for g in range(G):
            nc.vector.tensor_scalar_mul(out=acc[:][:, g, :],
                                        in0=gtv[:, g, 0, :],
                                        scalar1=xt[:][:, g:g + 1])
            for d in range(1, WROWS):
                nc.vector.scalar_tensor_tensor(
                    out=acc[:][:, g, :], in0=gtv[:, g, d, :],
                    scalar=xt[:][:, g:g + 1], in1=acc[:][:, g, :],
                    op0=mybir.AluOpType.mult, op1=mybir.AluOpType.add)
            eng = nc.sync if g % 2 == 0 else nc.scalar
            eng.dma_start(out=outv[:, g, :], in_=acc[:][:, g, :])
        return

    # ---- merged Cox-de Boor over all G groups ----
    # xmt[j] = x - t[i0+j]; ind8[j] = (x >= t[i0+j]); B0[j] = ind8[j]-ind8[j+1]
    xb8 = xt[:].to_broadcast([P, G, WKNOTS])
    xmt = sb.tile([P, G, WKNOTS], F32, tag="xmt")
    nc.vector.tensor_tensor(out=xmt[:], in0=xb8, in1=kwv,
                            op=mybir.AluOpType.subtract)
    ind8 = sb.tile([P, G, WKNOTS], F32, tag="ind8")
    nc.vector.tensor_tensor(out=ind8[:], in0=xb8, in1=kwv,
                            op=mybir.AluOpType.is_ge)
    B0 = sb.tile([P, G, WKNOTS - 1], F32, tag="B0")
    nc.vector.tensor_tensor(out=B0[:], in0=ind8[:][:, :, 0:WKNOTS - 1],
                            in1=ind8[:][:, :, 1:WKNOTS],
                            op=mybir.AluOpType.subtract)

    # batched divided differences D[g, kk, j] = t[g, j+kk+1] - t[g, j],
    # kk = 0..2 (level kk+1), j = 0..6. hi reads kw flat offsets
    # g*8 + kk + 1 + j <= 33 < KWPAD (pad memset above; garbage lanes of
    # D/R/U are never consumed by the level ops).
    W1 = WKNOTS - 1
    kwf = kw[:]  # [P, KWPAD]
    D = sb.tile([P, G, K, W1], F32, tag="D")
    U = sb.tile([P, G, K, W1], F32, tag="U")
    R = sb.tile([P, G, K, W1], F32, tag="R")
    if batched_d:
        hi = _strided(kwf, [[WKNOTS, G], [1, K], [1, W1]], 1)
        lo = _strided(kwf, [[WKNOTS, G], [0, K], [1, W1]], 0)
        nc.vector.tensor_tensor(out=D[:], in0=hi, in1=lo,
                                op=mybir.AluOpType.subtract)
        nc.vector.reciprocal(out=R[:].rearrange("p g k w -> p (g k w)"),
                             in_=D[:].rearrange("p g k w -> p (g k w)"))
        # U[g, kk, j] = xmt[g, j] * R[g, kk, j]
        xmt_b = _strided(xmt[:], [[WKNOTS, G], [0, K], [1, W1]], 0)
        nc.vector.tensor_tensor(out=U[:], in0=xmt_b, in1=R[:],
                                op=mybir.AluOpType.mult)
    else:
        for kk in range(K):
            # hi[p, g, j] = kw[p, g*8 + kk + 1 + j], j = 0..6
            hi = kwf[:, kk + 1:kk + 1 + G * WKNOTS] \
                .rearrange("p (g w) -> p g w", w=WKNOTS)[:, :, 0:W1]
            nc.vector.tensor_tensor(out=D[:][:, :, kk, :], in0=hi,
                                    in1=kwv[:, :, 0:W1],
                                    op=mybir.AluOpType.subtract)
        nc.vector.reciprocal(out=R[:].rearrange("p g k w -> p (g k w)"),
                             in_=D[:].rearrange("p g k w -> p (g k w)"))
        for kk in range(K):
            nc.vector.tensor_tensor(out=U[:][:, :, kk, :],
                                    in0=xmt[:][:, :, 0:W1],
                                    in1=R[:][:, :, kk, :],
                                    op=mybir.AluOpType.mult)

    # levels: Bk[i] = a[i] + (B[i+1] - a[i+1]),  a = U[kk-1] .* B (width L+1)
    prev = B0
    for kk in range(1, K + 1):
        L = WKNOTS - 1 - kk
        a = sb.tile([P, G, L + 1], F32, tag=f"a{kk}")
        nc.vector.tensor_tensor(out=a[:], in0=U[:][:, :, kk - 1, 0:L + 1],
                                in1=prev[:][:, :, 0:L + 1],
                                op=mybir.AluOpType.mult)
        t2 = sb.tile([P, G, L], F32, tag=f"t2{kk}")
        nc.vector.tensor_tensor(out=t2[:], in0=prev[:][:, :, 1:L + 1],
                                in1=a[:][:, :, 1:L + 1],
                                op=mybir.AluOpType.subtract)
        nxt = sb.tile([P, G, L], F32, tag=f"lvl{kk}")
        nc.vector.tensor_tensor(out=nxt[:], in0=a[:][:, :, 0:L],
                                in1=t2[:], op=mybir.AluOpType.add)
        prev = nxt
    wts = prev  # [P, G, 4]

    # ---- weighted sum of the 4 gathered rows, per group ----
    gtv = gt[:].rearrange("p g (d c) -> p g d c", d=WROWS)
    outv = out.rearrange("(p g) c -> p g c", g=G)
    if debug_dump == "i0":
        nc.sync.dma_start(out=outv[:, :, 0], in_=i0f[:])
        return
    if debug_dump == "kw":
        nc.sync.dma_start(out=outv[:, :, 0:WKNOTS], in_=kwv)
        return
    if debug_dump == "wts":
        nc.sync.dma_start(out=outv[:, :, 0:WROWS], in_=wts[:])
        return
    if debug_dump == "gt":
        nc.sync.dma_start(out=outv[:, :, 0:C], in_=gtv[:, :, 0, :])
        return
    if debug_dump == "gidx":
        gxf = sb.tile([P, G], F32, tag="gxf")
        nc.vector.tensor_copy(out=gxf[:], in_=gidx[:])
        nc.sync.dma_start(out=outv[:, :, 0], in_=gxf[:])
        return
    acc = sb.tile([P, G, C], F32, tag="acc")
    A = act_groups
    GA = G - A  # groups on the DVE STT chain
    for g in range(GA):
        nc.vector.tensor_scalar_mul(out=acc[:][:, g, :],
                                    in0=gtv[:, g, 0, :],
                                    scalar1=wts[:][:, g, 0:1])
        for d in range(1, WROWS):
            nc.vector.scalar_tensor_tensor(
                out=acc[:][:, g, :], in0=gtv[:, g, d, :],
                scalar=wts[:][:, g, d:d + 1], in1=acc[:][:, g, :],
                op0=mybir.AluOpType.mult, op1=mybir.AluOpType.add)
        eng = nc.sync if g % 2 == 0 else nc.scalar
        eng.dma_start(out=outv[:, g, :], in_=acc[:][:, g, :])
    if A:
        # trailing groups: multiplies on ACT, adds folded across groups on DVE
        prod = sb.tile([P, A, WROWS, C], F32, tag="prod", name="prod")
        for g in range(GA, G):
            for d in range(WROWS):
                nc.scalar.activation(out=prod[:][:, g - GA, d, :],
                                     in_=gtv[:, g, d, :],
                                     func=mybir.ActivationFunctionType.Copy,
                                     scale=wts[:][:, g, d:d + 1])
        pv = prod[:]
        nc.vector.tensor_tensor(out=pv[:, :, 0, :], in0=pv[:, :, 0, :],
                                in1=pv[:, :, 1, :], op=mybir.AluOpType.add)
        nc.vector.tensor_tensor(out=pv[:, :, 2, :], in0=pv[:, :, 2, :],
                                in1=pv[:, :, 3, :], op=mybir.AluOpType.add)
        nc.vector.tensor_tensor(out=acc[:][:, GA:G, :], in0=pv[:, :, 0, :],
                                in1=pv[:, :, 2, :], op=mybir.AluOpType.add)
        for g in range(GA, G):
            eng = nc.sync if g % 2 == 0 else nc.scalar
            eng.dma_start(out=outv[:, g, :], in_=acc[:][:, g, :])


def _declare(nc):
    coef = nc.dram_tensor("coefficients", [BC, N, C], F32, kind="ExternalInput")
    knots = nc.dram_tensor("knots", [BC, T], F32, kind="ExternalInput")
    inpce = nc.dram_tensor("inpce", [BC, 1], F32, kind="ExternalInput")
    out = nc.dram_tensor("out", [BC, C], F32, kind="ExternalOutput")
    return coef, knots, inpce, out


def build_nc(reps=1, bufs=1, **flags):
    cfg = {**DEFAULTS, **flags}
    nc = bacc.Bacc("TRN2", target_bir_lowering=False, debug=False,
                   num_devices=NCORES)
    coef, knots, inpce, out = _declare(nc)
    with tile.TileContext(nc) as tc:
        with tc.tile_pool(name="hoist", bufs=1) as hp, \
             tc.tile_pool(name="sb", bufs=bufs) as sb:
            hoisted = _emit_hoisted(tc, nc, hp)
            for _ in range(reps):
                _emit(tc, nc, sb, hoisted, coef.ap(), knots.ap(),
                      inpce.ap(), out.ap(), **cfg)
    nc.compile()
    return nc


def build_nc_loop(trip, unroll=16, bufs=3, pool_inside=False, **flags):
    cfg = {**DEFAULTS, **flags}
    nc = bacc.Bacc("TRN2", target_bir_lowering=False, debug=False,
                   num_devices=NCORES)
    coef, knots, inpce, out = _declare(nc)
    with tile.TileContext(nc) as tc:
        if pool_inside:
            with tc.For_i(0, trip, 1):
                for _ in range(unroll):
                    _emit(tc, nc, None, None, coef.ap(), knots.ap(),
                          inpce.ap(), out.ap(), **cfg)
        else:
            with tc.tile_pool(name="hoist", bufs=1) as hp, \
                 tc.tile_pool(name="sb", bufs=bufs) as sb:
                hoisted = _emit_hoisted(tc, nc, hp)
                with tc.For_i(0, trip, 1):
                    for _ in range(unroll):
                        _emit(tc, nc, sb, hoisted, coef.ap(), knots.ap(),
                              inpce.ap(), out.ap(), **cfg)
    nc.compile()
    return nc


_NC_CACHE = None


def kernel(coefficients, knots, inpce, **run_kwargs):
    global _NC_CACHE
    if _NC_CACHE is None:
        _NC_CACHE = build_nc()
    nc = _NC_CACHE
    coefficients = np.ascontiguousarray(coefficients, dtype=np.float32)
    knots = np.ascontiguousarray(knots, dtype=np.float32)
    inpce = np.ascontiguousarray(inpce, dtype=np.float32)
    in_maps = []
    for k in range(NCORES):
        s = slice(k * BC, (k + 1) * BC)
        in_maps.append({"coefficients": coefficients[s],
                        "knots": knots[s],
                        "inpce": inpce[s]})
    res = run_bass_kernel_spmd(nc, in_maps, core_ids=list(range(NCORES)),
                               **run_kwargs)
    out = np.concatenate([res.results[k]["out"] for k in range(NCORES)], axis=0)
    if run_kwargs:
        return out, res
    return out
